# revision 1
# baseline (speedup 1.0000x reference)
"""Causal self-attention head with pairwise-MLP scoring, on 8 trn2 cores.

Math (per batch b):
  q = relu(x Wq + bq); k = relu(x Wk + bk); v = x Wv + bv
  s[q,k] = W2 . relu(qp[q] + kp[k] + b1) + b2,  qp = q W1[:D], kp = k W1[D:]
  out = softmax(causal(s)) @ v          (b2 drops out of the softmax)

Sharding: 16 query chunks of 128 rows (2 batches x 8 chunks). Core c gets
batch c//4 and the balanced causal pair (j, 7-j), j = c%4: the "short"
chunk j needs keys [0, 512), the "long" chunk 7-j needs keys [0, 1024).
Every core runs the identical static program; causality comes from an
additive -inf mask prepared on the host (folded into the psum->SBUF
evacuation of the score blocks).

Score computation: an h-tile [128, W] holds relu(kpb[d, k] + qp[d, q])
for 2 queries stacked on the partition dim (2 x 64 d-lanes); a float32r
matmul against a [128, 64] two-column weight view reduces d, emitting 2
score rows per pair. f32r matmuls must write psum partition 0, so pairs
accumulate in [64, 512] psum blocks (32 pairs each) that are evacuated
to the [128, w] SBUF scores tile with partition-shifting DVE adds that
also apply the causal mask. The 32 weight views are overlapping slices
of one [128, 126] tensor (w2 pinned at absolute columns 62/63). Queries
are paired (q, q+64); the row permutation is undone on the host.
h production is split across DVE, ACT and GPSIMD.
"""

import numpy as np

B, T, E, D = 2, 1024, 256, 64
NCORES = 8
NEG = -1.0e30

_compiled = None
_last_results = None


def _build_bass():
    import concourse.bacc as bacc
    import concourse.tile as tile
    import concourse.mybir as mybir
    from concourse.masks import make_identity

    f32 = mybir.dt.float32
    f32r = mybir.dt.float32r
    nc = bacc.Bacc(None, target_bir_lowering=False)

    bigr_d = nc.dram_tensor("bigr", [128, 3072], f32r, kind="ExternalInput")
    bigf_d = nc.dram_tensor("bigf", [128, 1664], f32, kind="ExternalInput")
    brow_d = nc.dram_tensor("brow", [4, 1, D], f32r, kind="ExternalInput")
    out_d = nc.dram_tensor("out", [2, 128, D], f32, kind="ExternalOutput")

    with tile.TileContext(nc) as tc:
        with (
            tc.tile_pool(name="singles", bufs=1) as singles,
            tc.tile_pool(name="hpool", bufs=10) as hpool,
            tc.tile_pool(name="epool", bufs=2) as epool,
            tc.tile_pool(name="etp", bufs=4) as etp,
            tc.tile_pool(name="ps_proj", bufs=3, space="PSUM") as ps_proj,
            tc.tile_pool(name="ps_sc", bufs=3, space="PSUM") as ps_sc,
            tc.tile_pool(name="ps_tr", bufs=1, space="PSUM") as ps_tr,
            tc.tile_pool(name="ps_o", bufs=1, space="PSUM") as ps_o,
        ):
            AF = mybir.ActivationFunctionType
            OP = mybir.AluOpType

            # ---- two packed inputs (f32r: x + q/k-side weights; f32:
            # wv + masks); SP trigger is ~650ns per dma_start so few big
            # transfers, ordered by first use ----
            # bigr cols: 0:512 xtq | 512:640 wq | 640:704 w1a | 704:832 wk
            #  | 832:896 w1b | 896:1022 w64 | 1022:3070 xt (kb*1024+ec*512)
            # bigf cols: 0:128 wv | 128:640 mask_s | 640:1664 mask_l
            bigr_sb = singles.tile([128, 3072], f32r, tag="bigr")
            bigf_sb = singles.tile([128, 1664], f32, tag="bigf")
            brow_sb = singles.tile([1, 4 * D], f32r, tag="brow")
            nc.sync.dma_start(out=brow_sb,
                              in_=brow_d[:].rearrange("a b d -> b (a d)"))
            nc.sync.dma_start(out=bigr_sb[:, 0:1022], in_=bigr_d[:, 0:1022])
            nc.sync.dma_start(out=bigr_sb[:, 1022:2046], in_=bigr_d[:, 1022:2046])
            nc.sync.dma_start(out=bigr_sb[:, 2046:3070], in_=bigr_d[:, 2046:3070])
            nc.sync.dma_start(out=bigf_sb, in_=bigf_d[:])
            xtq_sb = bigr_sb[:, 0:512]
            wq_sb = bigr_sb[:, 512:640]
            w1a_sb = bigr_sb[0:D, 640:704]
            wk_sb = bigr_sb[:, 704:832]
            w1b_sb = bigr_sb[0:D, 832:896]
            w64_sb = bigr_sb[:, 896:1022]
            XT0 = 1022
            wv_sb = bigf_sb[:, 0:128]
            mask_s_sb = bigf_sb[:, 128:640]
            mask_l_sb = bigf_sb[:, 640:1664]

            ones_f = singles.tile([1, 512], f32, tag="ones_f")
            nc.vector.memset(ones_f, 1.0)
            ones_row = singles.tile([1, 512], f32r, tag="ones_row")
            nc.vector.tensor_copy(ones_row, ones_f)
            ident = singles.tile([128, 128], f32, tag="ident")
            make_identity(nc, ident)

            # preload the exp table set early
            warm = singles.tile([128, 1], f32, tag="warm")
            nc.vector.memset(warm, 0.0)
            nc.scalar.activation(warm, warm, AF.Exp)

            bq_row = brow_sb[:, 0 * D:1 * D]
            bk_row = brow_sb[:, 1 * D:2 * D]
            b1_row = brow_sb[:, 2 * D:3 * D]
            bv_row = brow_sb[:, 3 * D:4 * D]

            import os as _os
            reps = int(_os.environ.get("K_REPS", "1"))
            for _rep in range(reps):
              # ---- projections, interleaved with the main pass so the
              # short-chunk pairwise work starts as early as possible ----
              q_ps = ps_proj.tile([D, 256], f32, tag="proj")
              for ec in range(2):
                  nc.tensor.matmul(q_ps,
                                   wq_sb[:, ec * D:(ec + 1) * D].bitcast(f32),
                                   xtq_sb[:, ec * 256:(ec + 1) * 256].bitcast(f32),
                                   start=(ec == 0), stop=False)
              nc.tensor.matmul(q_ps, bq_row.bitcast(f32), ones_f[:, 0:256],
                               start=False, stop=True)
              q_sb = singles.tile([D, 256], f32, tag="q")
              nc.scalar.activation(q_sb, q_ps, AF.Relu)

              qp_ps = ps_proj.tile([D, 256], f32, tag="proj")
              nc.tensor.matmul(qp_ps, w1a_sb.bitcast(f32), q_sb,
                               start=True, stop=True)
              qp2 = singles.tile([128, 128], f32, tag="qp2")
              for ch in range(2):          # 0=short,1=long chunk
                  for h in range(2):       # partition half = query p, p+64
                      nc.scalar.activation(
                          qp2[h * D:(h + 1) * D, ch * D:(ch + 1) * D],
                          qp_ps[:, ch * 128 + h * D: ch * 128 + (h + 1) * D],
                          AF.Copy)

              k_sb = singles.tile([D, T], f32, tag="k")
              kpb2a = singles.tile([128, 512], f32, tag="kpb2a")
              kpb2b = singles.tile([128, 512], f32, tag="kpb2b")
              kpb2 = [kpb2a, kpb2b]

              def k_chain(kb):
                  k_ps = ps_proj.tile([D, 512], f32, tag="proj")
                  for ec in range(2):
                      nc.tensor.matmul(
                          k_ps, wk_sb[:, ec * D:(ec + 1) * D].bitcast(f32),
                          bigr_sb[:, XT0 + kb * 1024 + ec * 512:
                                  XT0 + kb * 1024 + (ec + 1) * 512].bitcast(f32),
                          start=(ec == 0), stop=False)
                  nc.tensor.matmul(k_ps, bk_row.bitcast(f32), ones_f,
                                   start=False, stop=True)
                  nc.scalar.activation(k_sb[:, kb * 512:(kb + 1) * 512], k_ps, AF.Relu)
                  kp_ps = ps_proj.tile([D, 512], f32, tag="proj")
                  nc.tensor.matmul(kp_ps, w1b_sb.bitcast(f32),
                                   k_sb[:, kb * 512:(kb + 1) * 512],
                                   start=True, stop=False)
                  nc.tensor.matmul(kp_ps, b1_row.bitcast(f32), ones_f,
                                   start=False, stop=True)
                  for h in range(2):
                      nc.scalar.activation(kpb2[kb][h * D:(h + 1) * D, :],
                                           kp_ps, AF.Copy)

              k_chain(0)
              k_chain(1)

              # ---- main pairwise pass ----
              ssb_s = singles.tile([128, 512], f32, tag="ssb_s")
              ssb_l = singles.tile([128, 1024], f32, tag="ssb_l")

              sched_state = [0]
              import os as _os2
              mix = tuple(int(v) for v in _os2.environ.get("HMIX", "8,4,3").split(","))

              def h_op(h2slice, kb, col):
                  i = sched_state[0]
                  sched_state[0] += 1
                  d, a, p = mix
                  r = i % (d + a + p)
                  if r < d:
                      nc.vector.tensor_scalar(h2slice, kpb2[kb], col, 0.0,
                                              OP.add, OP.max)
                  elif r < d + a:
                      nc.scalar.activation(h2slice, kpb2[kb], AF.Relu, bias=col)
                  else:
                      nc.gpsimd.tensor_scalar(h2slice, kpb2[kb], col, 0.0,
                                              OP.add, OP.max)

              def w64v(sub):
                  return w64_sb[:, 62 - 2 * sub: 126 - 2 * sub]

              def score_block_short(ssb, qcol, moff):
                  for blk in range(2):
                      sub_ps = ps_sc.tile([64, 512], f32, tag="scsub")
                      for sub in range(32):
                          p = 32 * blk + sub
                          h2 = hpool.tile([128, 512], f32r, tag="h2")
                          h_op(h2, 0, qp2[:, qcol + p: qcol + p + 1])
                          nc.tensor.matmul(sub_ps, w64v(sub), h2,
                                           start=(sub == 0), stop=(sub == 31))
                      rows = slice(64 * blk, 64 * (blk + 1))
                      nc.vector.tensor_tensor(
                          ssb[rows, 0:512], sub_ps,
                          bigf_sb[rows, moff:moff + 512], OP.add)

              def score_block_long(ssb, qcol, moff):
                  for blk in range(2):
                      ps_a = ps_sc.tile([64, 512], f32, tag="scsub")
                      ps_b = ps_sc.tile([64, 512], f32, tag="scsub")
                      for sub in range(32):
                          p = 32 * blk + sub
                          h2 = hpool.tile([128, 1024], f32r, tag="h2w")
                          col = qp2[:, qcol + p: qcol + p + 1]
                          h_op(h2[:, 0:512], 0, col)
                          h_op(h2[:, 512:1024], 1, col)
                          nc.tensor.matmul(ps_a, w64v(sub), h2[:, 0:512],
                                           start=(sub == 0), stop=(sub == 31))
                          nc.tensor.matmul(ps_b, w64v(sub), h2[:, 512:1024],
                                           start=(sub == 0), stop=(sub == 31))
                      rows = slice(64 * blk, 64 * (blk + 1))
                      nc.vector.tensor_tensor(
                          ssb[rows, 0:512], ps_a,
                          bigf_sb[rows, moff:moff + 512], OP.add)
                      nc.vector.tensor_tensor(
                          ssb[rows, 512:1024], ps_b,
                          bigf_sb[rows, moff + 512:moff + 1024], OP.add)

              def epilogue(ssb, width, slot):
                  exp_sb = epool.tile([128, 1024], f32, tag="exp")
                  nhalf = width // 512
                  l_parts = []
                  for hh in range(nhalf):
                      l_sb = etp.tile([128, 1], f32, tag="l")
                      nc.scalar.activation(exp_sb[:, hh * 512:(hh + 1) * 512],
                                           ssb[:, hh * 512:(hh + 1) * 512],
                                           AF.Exp, accum_out=l_sb)
                      l_parts.append(l_sb)
                  if nhalf == 2:
                      l_tot = etp.tile([128, 1], f32, tag="lt")
                      nc.vector.tensor_tensor(l_tot, l_parts[0], l_parts[1], OP.add)
                  else:
                      l_tot = l_parts[0]
                  rl_sb = etp.tile([128, 1], f32, tag="rl")
                  nc.vector.reciprocal(rl_sb, l_tot)
                  o_ps = ps_o.tile([128, D], f32, tag="o")
                  nkc = width // 128
                  for kc in range(nkc):
                      tr_ps = ps_tr.tile([128, 128], f32, tag="tr")
                      nc.tensor.transpose(tr_ps, exp_sb[:, kc * 128:(kc + 1) * 128],
                                          ident)
                      et_sb = etp.tile([128, 128], f32, tag="et")
                      if kc % 2 == 0:
                          nc.vector.tensor_copy(et_sb, tr_ps)
                      else:
                          nc.scalar.activation(et_sb, tr_ps, AF.Copy)
                      nc.tensor.matmul(o_ps, et_sb, v_sb[:, kc * D:(kc + 1) * D],
                                       start=(kc == 0), stop=(kc == nkc - 1))
                  o_sb = etp.tile([128, D], f32, tag="osb")
                  nc.vector.tensor_scalar(o_sb, o_ps, rl_sb, None, OP.mult)
                  nc.sync.dma_start(out=out_d[slot], in_=o_sb)

              score_block_short(ssb_s, 0, 128)

              # v, chunk-column layout: chunk kc at cols kc*64
              v_sb = singles.tile([128, 8 * D], f32, tag="v")
              for kc in range(8):
                  v_ps = ps_proj.tile([128, D], f32, tag="proj")
                  for ec in range(2):
                      c0 = XT0 + (kc // 4) * 1024 + ec * 512 + (kc % 4) * 128
                      nc.tensor.matmul(v_ps, bigr_sb[:, c0:c0 + 128].bitcast(f32),
                                       wv_sb[:, ec * D:(ec + 1) * D],
                                       start=(ec == 0), stop=False)
                  nc.tensor.matmul(v_ps, ones_f[:, 0:128],
                                   bv_row.bitcast(f32),
                                   start=False, stop=True)
                  if kc % 2 == 0:
                      nc.scalar.activation(v_sb[:, kc * D:(kc + 1) * D], v_ps, AF.Copy)
                  else:
                      nc.vector.tensor_copy(v_sb[:, kc * D:(kc + 1) * D], v_ps)

              epilogue(ssb_s, 512, 0)
              score_block_long(ssb_l, 64, 640)
              epilogue(ssb_l, 1024, 1)

    nc.compile()
    return nc


def kernel(x, Wq, bq, Wk, bk, Wv, bv, W1, b1, W2, b2):
    global _compiled, _last_results
    import os
    from concourse.bass_utils import run_bass_kernel_spmd

    x = np.asarray(x, np.float32)
    W1a, W1b = np.ascontiguousarray(W1[:D]), np.ascontiguousarray(W1[D:])
    w64 = np.zeros((128, 126), np.float32)
    w64[0:D, 62] = W2[:, 0]
    w64[D:128, 63] = W2[:, 0]
    brow = np.stack([bq.reshape(1, D), bk.reshape(1, D),
                     b1.reshape(1, D), bv.reshape(1, D)]).astype(np.float32)

    # device scores row r <-> chunk query perm[r]
    r = np.arange(128)
    perm = 32 * (r // 64) + (r % 64) // 2 + 64 * (r % 2)

    in_maps = []
    for c in range(NCORES):
        b, j = divmod(c, 4)
        chunks = (j, 7 - j)  # (short, long)
        bigr = np.zeros((128, 3072), np.float32)
        bigf = np.zeros((128, 1664), np.float32)
        xtb = x[b].T  # [E=256, T]
        xtq = np.concatenate(
            [xtb[:, ch * 128:(ch + 1) * 128] for ch in chunks], axis=1)
        for ec in range(2):
            bigr[:, ec * 256:(ec + 1) * 256] = xtq[ec * 128:(ec + 1) * 128, :]
        bigr[:, 512:576] = Wq[0:128]
        bigr[:, 576:640] = Wq[128:256]
        bigr[0:D, 640:704] = W1a
        bigr[:, 704:768] = Wk[0:128]
        bigr[:, 768:832] = Wk[128:256]
        bigr[0:D, 832:896] = W1b
        bigr[:, 896:1022] = w64
        for kb in range(2):
            for ec in range(2):
                c0 = 1022 + kb * 1024 + ec * 512
                bigr[:, c0:c0 + 512] = xtb[ec * 128:(ec + 1) * 128,
                                           kb * 512:(kb + 1) * 512]
        bigf[:, 0:64] = Wv[0:128]
        bigf[:, 64:128] = Wv[128:256]
        for (ch, width, c0) in ((chunks[0], 512, 128), (chunks[1], 1024, 640)):
            gq = ch * 128 + perm
            kk = np.arange(width)
            bigf[:, c0:c0 + width] = np.where(kk[None, :] <= gq[:, None],
                                              0.0, NEG)
        in_maps.append({"bigr": bigr, "bigf": bigf, "brow": brow})

    if _compiled is None:
        _compiled = _build_bass()

    trace = os.environ.get("KTRACE", "0") == "1"
    res = run_bass_kernel_spmd(_compiled, in_maps, list(range(NCORES)),
                               trace=trace)
    _last_results = res
    outs = res.results

    inv = np.argsort(perm)
    y = np.empty((B, T, D), np.float32)
    for c in range(NCORES):
        b, j = divmod(c, 4)
        o = np.asarray(outs[c]["out"])
        for slot, ch in enumerate((j, 7 - j)):
            y[b, ch * 128:(ch + 1) * 128] = o[slot][inv]
    return y

